# revision 59
# baseline (speedup 1.0000x reference)
"""Causal self-attention with RoPE on 8 trn2 NeuronCores.

Full inputs -> full output. Sharding: data-parallel on batch (2) x
tensor-parallel on heads (4 heads/core). Each core computes qkv projections,
RoPE, causal attention for its 4 heads, and a partial output projection
(row-parallel slice of Wproj); the host sums the 4 partials per batch.

Per-core bass kernel layout choices:
  - Q/K are produced transposed ([head_dim, T]) directly by the projection
    matmuls, so scores S^T[k, q] = K_roped^T(lhsT) x Q_roped(rhs) needs no
    on-device transposes anywhere.
  - RoPE: rot(q) = R @ q for a fixed 128x128 block-diag permutation/sign
    matrix on the tensor engine. Since sin/cos are constant within each
    even/odd pair, rot(q*sin) == rot(q)*sin, so the sin-multiply runs
    before the rotation matmul and the vector engine does q*cos + psum-add.
    sin/cos tables are bf16 (halves their prologue DMA bytes).
  - softmax skips max-subtraction (scores are ~N(0,1); the diagonal
    |q|^2/8 term is <= ~15, exp stays well inside fp32/bf16 range) and gets
    the denominator for free from an appended ones-column in V.
  - all matmuls run in bf16 (halves PE weight-load time + DMA bytes vs
    f32r; rel err ~3.9e-3, well under the 2e-2 budget).
  - softmax denominators: stream-transpose the psum row into partitions
    (32x32 DVE blocks), reciprocal on the strided elems/partition,
    transpose back to a single row, then GPSIMD partition_broadcast fans
    the recip row out to all 128 partitions SBUF->SBUF (a DRAM round-trip
    here would serialize the sync DMA queue and put ~4.5us of latency on
    the normalize critical path).
  - AV matmuls are emitted av_defer (=7) kt-blocks behind their scores so
    the in-order PE never waits on the scalar engine's exp chain - the exp
    stream is the per-block pacer in the long attention loops.
  - causality: k-tile x q-column blocks above the diagonal are skipped at
    128-column granularity; diagonal 128x128 blocks are masked post-exp.
  - all DRAM tensors are laid out host-side to exactly match their SBUF
    tiles, so every load/store is a handful of large fully-contiguous
    DMAs (each HWDGE issue holds the shared DGE ~630ns; a fine-grained
    schedule burns tens of us of serialized issue time). Half the x loads
    go through the pool-queue SWDGE path, which bypasses the shared HWDGE
    entirely.
  - y is written back via a per-chunk staging tile -> one DMA per chunk.
  - emission is software-pipelined at head-pair granularity (see the
    schedule at the bottom of _build_nc): A-projection and C-out-proj
    steps weave into the attention block loops with per-step deadlines,
    and the last pair's normalize + output projection are merged and
    pipelined per 128-column q tile so the endgame latency chain
    (normalize -> broadcast -> out-proj -> writeback) overlaps itself.
  - schedule knobs (DEFAULT_SCHED) were tuned against the TimelineSim
    cost model, which tracked measured HW within a few percent.
"""

import math
from functools import lru_cache

import numpy as np

import concourse.bacc as bacc
import concourse.bass as bass
import concourse.mybir as mybir
import concourse.tile as tile
from concourse.bass import ds, ts
from concourse.bass_utils import run_bass_kernel_spmd

B, T, C = 2, 2048, 1024
H, DH = 16, 64
HL = 4  # heads per core
NCORES = 8
ROPE_BASE = 10000.0

F32 = mybir.dt.float32
F32R = mybir.dt.float32r
BF16 = mybir.dt.bfloat16
AF = mybir.ActivationFunctionType
ALU = mybir.AluOpType

TCH = 512  # T chunk (free dim of projection / q chunk of attention)
NJ = T // TCH  # 4
NKT = T // 128  # 16 k tiles


DEFAULT_SCHED = {
    "a20_b11": 2,  # steps of A(2,0)'s tail pulled forward into B(1,1)
    "a30_b21": 2,  # steps of A(3,0)'s tail pulled forward into B(2,1)
    "c1_b21": 0,   # C(1) steps woven into B(2,1) (rest -> B(3,0))
    "c2_b31": 9,   # C(2) steps woven into B(3,1) (rest -> B(0,0))
    "c3_b00": 2,   # C(3) steps woven into B(0,0) (rest -> finale)
    "av_defer": 7,  # blocks of slack the exp stream gets before its AV
    "av_defer0": 2,  # same, for chunk 0's short (4-block) pairs
}


def _build_nc(reps=1, variant="", sched=None):
    s = dict(DEFAULT_SCHED)
    if sched:
        s.update(sched)
    nc = bacc.Bacc("TRN2", target_bir_lowering=False, debug=False, num_devices=NCORES)

    # DRAM layouts are SBUF-tile-exact (see _in_maps): contiguous big DMAs.
    xt_d = nc.dram_tensor("xt", [128, NJ, 8, TCH], BF16, kind="ExternalInput")
    # pack-major: [:, pk] is one contiguous per-pack slab (Q0, Q1, K0, K1)
    wqk = nc.dram_tensor("wqk", [128, 4, 8, 128], BF16, kind="ExternalInput")
    wv = nc.dram_tensor("wv", [128, 8, 256], BF16, kind="ExternalInput")
    wpj = nc.dram_tensor("wpj", [128, 2, 1024], BF16, kind="ExternalInput")
    rm = nc.dram_tensor("rm", [128, 2, 128], BF16, kind="ExternalInput")
    cosp = nc.dram_tensor("cosp", [128, T], BF16, kind="ExternalInput")
    sinp = nc.dram_tensor("sinp", [128, T], BF16, kind="ExternalInput")
    y = nc.dram_tensor("y", [128, NJ, 4, 2, 512], BF16, kind="ExternalOutput")

    with tile.TileContext(nc) as tc:
        with (
            tc.tile_pool(name="const", bufs=1) as const,
            tc.tile_pool(name="persist", bufs=1) as persist,
            tc.tile_pool(name="work", bufs=2) as work,
            tc.tile_pool(name="expool", bufs=9) as expool,
            tc.tile_pool(name="small", bufs=2) as small,
            tc.tile_pool(name="outst", bufs=2) as outst,
            tc.tile_pool(name="psA", bufs=2, space="PSUM") as psA,
            tc.tile_pool(name="psS", bufs=2, space="PSUM") as psS,
            tc.tile_pool(name="psV", bufs=2, space="PSUM") as psV,
        ):
            # ---- constants ----
            wqk_sb = const.tile([128, 4, 8, 128], BF16)
            wv_sb = const.tile([128, 8, 256], BF16)
            wpj_sb = const.tile([128, 2, 1024], BF16)
            rm_sb = const.tile([128, 2, 128], BF16)  # [:,0]=r2t, [:,1]=mask
            cos_sb = const.tile([128, T], BF16)
            sin_sb = const.tile([128, T], BF16)
            r2t_sb = rm_sb[:, 0, :]
            mask_sb = rm_sb[:, 1, :]

            # ---- persistent activations ----
            xt_all = persist.tile([128, NJ, 8, TCH], BF16, tag="xt_all")
            qt_packs = [
                persist.tile([128, T], BF16, tag=f"qt{p}", name=f"qt{p}")
                for p in range(2)
            ]
            kt_packs = [
                persist.tile([128, T], BF16, tag=f"kt{p}", name=f"kt{p}")
                for p in range(2)
            ]
            # V for even local heads: [.., 96] = [v dims | ones | zeros x31]
            # (96 so the AV psum output covers partitions 64:96, keeping the
            # denominator stream-transpose window fully initialized)
            vse = persist.tile([128, NKT, 2, 96], BF16, tag="vse")
            # V for odd local heads: [.., 128] = [ones | zeros x63 | v dims]
            vso = persist.tile([128, NKT, 2, 128], BF16, tag="vso")
            ytsb = persist.tile([128, 2, T], BF16, tag="ytsb")

            def gen_A_pp(j, pp):
                """Projections + RoPE for chunk j, head-pair pp (Q/K packs
                pp and pp+2); pair 0 also emits the V tiles. Yields between
                groups so the steps weave into attention block loops."""
                xt = xt_all[:, j]

                def emit_rot(t1, tsin, pk):
                    rp = psA.tile([128, TCH], F32, tag="mm", name=f"rp{j}{pk}")
                    nc.tensor.matmul(rp[:], r2t_sb, tsin[:], start=True, stop=True)
                    dest = (qt_packs + kt_packs)[pk]
                    nc.vector.tensor_tensor(dest[:, ts(j, TCH)], t1[:], rp[:], ALU.add)

                def emit_v(i):
                    kt_i = j * 4 + i
                    pv = psA.tile([128, 256], F32, tag="mm", name=f"pv{j}{i}")
                    for cc in range(8):
                        nc.tensor.matmul(
                            pv[:],
                            xt[:, cc, ts(i, 128)],
                            wv_sb[:, cc, :],
                            start=(cc == 0),
                            stop=(cc == 7),
                        )
                    pv4 = pv[:].rearrange("p (h d) -> p h d", h=4)
                    nc.any.tensor_copy(vse[:, kt_i, :, 0:64], pv4[:, 0::2, :])
                    nc.any.tensor_copy(vso[:, kt_i, :, 64:128], pv4[:, 1::2, :])

                # V-group MMs interleave between packs: they fill the PE
                # while the DVE rope chain consumes the previous pack's psum
                for g, pk in enumerate((pp, pp + 2)):  # Q pack pp, K pack pp
                    ps = psA.tile([128, TCH], F32, tag="mm", name=f"ps{j}{pk}")
                    for cc in range(8):
                        nc.tensor.matmul(
                            ps[:],
                            wqk_sb[:, pk, cc, :],
                            xt[:, cc, :],
                            start=(cc == 0),
                            stop=(cc == 7),
                        )
                    # rot(q*sin) == rot(q)*sin: sin-multiply straight off
                    # PSUM; t1 right behind it so ps's psum slot frees after
                    # two back-to-back DVE ops - the V-group (or woven B
                    # work) covers that latency before the rot matmul needs
                    # tsin
                    tsin = work.tile([128, TCH], BF16, tag="tsin", name=f"tsn{j}{pk}")
                    nc.vector.tensor_tensor(
                        tsin[:], ps[:], sin_sb[:, ts(j, TCH)], ALU.mult
                    )
                    t1 = work.tile([128, TCH], F32, tag="t1", name=f"t1{j}{pk}")
                    nc.vector.tensor_tensor(
                        t1[:], ps[:], cos_sb[:, ts(j, TCH)], ALU.mult
                    )
                    yield
                    if pp == 0:
                        emit_v(2 * g)
                        yield
                    emit_rot(t1, tsin, pk)
                    yield
                    if pp == 0:
                        emit_v(2 * g + 1)
                        yield

            def emit_norm(j, pp, avE, avO, q0, qn, kalt=""):
                """Normalize columns [q0, q0+qn) of chunk j pair pp.

                Even head sums sit at avE row 64, odd head sums at avO row
                0. The row-wise DVE reciprocal is ~9 cyc/elem and
                free-dim-serial, so: stream-transpose the row into
                partitions (32x32 blocks), reciprocal on the strided
                elems/partition, transpose back to a single row, then
                GPSIMD partition_broadcast fans it out SBUF->SBUF.
                """
                cs = slice(q0, q0 + qn)
                tEO = small.tile([32, 2, qn], F32, tag=f"tEO{kalt}",
                                 name=f"tE{j}{pp}{q0}")
                nc.vector.transpose(tEO[:, 0, :], avE[64:96, cs])
                nc.vector.transpose(tEO[:, 1, :], avO[0:32, cs])
                tc_ = tEO.rearrange("p e (b s) -> p e b s", s=32)[:, :, :, 0]
                nc.vector.reciprocal(tc_, tc_)
                rEO = small.tile([32, 2, qn], F32, tag=f"rEO{kalt}",
                                 name=f"rE{j}{pp}{q0}")
                nc.vector.transpose(
                    rEO.rearrange("p e t -> p (e t)"),
                    tEO.rearrange("p e t -> p (e t)"),
                )
                tsl = ds(j * TCH + q0, qn)
                bc = small.tile([128, 2, qn], F32, tag=f"bc{kalt}",
                                name=f"bc{j}{pp}{q0}")
                nc.gpsimd.partition_broadcast(bc[:, :, :], rEO[0:1, :, :],
                                              channels=128)
                nc.vector.tensor_tensor(
                    ytsb[0:64, pp, tsl], avE[0:64, cs], bc[0:64, 0, :],
                    ALU.mult,
                )
                nc.vector.tensor_tensor(
                    ytsb[64:128, pp, tsl], avO[64:128, cs], bc[64:128, 1, :],
                    ALU.mult,
                )

            def gen_B_pp(j, pp, finale=False):
                """Attention for q chunk j, head pair pp. Yields between kt
                blocks. finale mode (last pair processed): the normalize is
                pipelined per 128-column q tile and the chunk's output
                projection is merged in per tile, so the endgame latency
                chain (normalize -> broadcast -> out-proj -> writeback)
                overlaps itself instead of serializing at full width."""
                KT = kt_packs[pp]
                QT = qt_packs[pp]
                n_kt = 4 * j + 4
                # 96 partitions so [64:96] is a valid aligned window for
                # the stream-transpose of the denominator row (row 64)
                avE = psV.tile([96, TCH], F32, tag="av", name=f"avE{j}{pp}")
                avO = psV.tile([128, TCH], F32, tag="av", name=f"avO{j}{pp}")

                def emit_av(kt, ex, col0):
                    st = kt == 0
                    sp = kt == n_kt - 1
                    nc.tensor.matmul(
                        avE[0:96, col0:], vse[:, kt, pp, :], ex[:, 0, col0:],
                        start=st, stop=sp,
                    )
                    nc.tensor.matmul(
                        avO[:, col0:], vso[:, kt, pp, :], ex[:, 1, col0:],
                        start=st, stop=sp,
                    )

                pend_av = []  # (kt, ex, col0): AV deferred av_defer blocks
                for kt in range(n_kt):
                    d = kt - 4 * j
                    col0 = max(d, 0) * 128
                    qsl = ds(j * TCH + col0, TCH - col0)
                    sc = psS.tile([128, 2, TCH], F32, tag="sc", name=f"sc{j}{pp}{kt}")
                    nc.tensor.matmul(
                        sc[:, 0, col0:],
                        KT[0:64, ts(kt, 128)],
                        QT[0:64, qsl],
                        start=True,
                        stop=True,
                        tile_position=(0, 0),
                    )
                    nc.tensor.matmul(
                        sc[:, 1, col0:],
                        KT[64:128, ts(kt, 128)],
                        QT[64:128, qsl],
                        start=True,
                        stop=True,
                        tile_position=(64, 0),
                    )
                    ex = expool.tile([128, 2, TCH], BF16, tag="ex", name=f"ex{j}{pp}{kt}")
                    if "noexp" not in variant:
                        nc.scalar.activation(
                            ex[:, :, col0:], sc[:, :, col0:], AF.Exp,
                            scale=1.0 / math.sqrt(DH),
                        )
                    if d >= 0:
                        msl = slice(col0, col0 + 128)
                        nc.vector.tensor_tensor(
                            ex[:, :, msl],
                            ex[:, :, msl],
                            mask_sb[:, None, :].to_broadcast((128, 2, 128)),
                            ALU.mult,
                        )
                    # emit a PREVIOUS block's AV matmuls here: by now
                    # its exp+mask have had av_defer full blocks of scalar
                    # time, so the in-order PE doesn't stall on them
                    pend_av.append((kt, ex, col0))
                    defer = s["av_defer"] if j else s["av_defer0"]
                    if len(pend_av) > defer:
                        emit_av(*pend_av.pop(0))
                    yield
                for p_ in pend_av:
                    emit_av(*p_)
                if not finale:
                    emit_norm(j, pp, avE, avO, 0, TCH)
                    yield
                    return
                # finale: per-qt normalize + merged output projection
                ost = outst.tile([128, 4, 2, 512], BF16, tag="ost", name=f"ost{j}")
                for i in range(4):
                    qt_i = j * 4 + i
                    # normalize two q tiles per chain: halves the serialized
                    # DVE chain overhead (the finale's binding resource)
                    if i % 2 == 0:
                        emit_norm(j, pp, avE, avO, i * 128, 256,
                                  kalt=str((i // 2) % 2))
                    # the cc=0 half reads only pair-0's ytsb (normalized long
                    # ago): emit both co tiles' cc=0 matmuls BEFORE this qt's
                    # normalize chain so the PE works while the DVE chain and
                    # broadcast run
                    pjs = []
                    for co in range(2):
                        # co=1 tiles borrow the sc ring's psum banks (the exp
                        # stream is drained here): 4 live slots instead of 2,
                        # so the next qt's matmuls never wait on this qt's
                        # staging copies
                        if co:
                            pj2 = psS.tile([128, 2, TCH], F32, tag="sc",
                                           name=f"pj{j}{i}{co}")
                            pj = pj2[:, 0, :]
                        else:
                            pj = psA.tile([128, TCH], F32, tag="mm",
                                          name=f"pj{j}{i}{co}")[:]
                        nc.tensor.matmul(
                            pj,
                            ytsb[:, 0, ts(qt_i, 128)],
                            wpj_sb[:, 0, ts(co, TCH)],
                            start=True,
                            stop=False,
                        )
                        pjs.append(pj)
                    for co in range(2):
                        nc.tensor.matmul(
                            pjs[co],
                            ytsb[:, 1, ts(qt_i, 128)],
                            wpj_sb[:, 1, ts(co, TCH)],
                            start=False,
                            stop=True,
                        )
                        # both copies on ACT: the exp stream is over and the
                        # DVE is saturated by the normalize chains here
                        nc.scalar.copy(ost[:, i, co, :], pjs[co])
                        nc.sync.dma_start(y[:, j, i, co], ost[:, i, co])
                    yield

            def gen_C(j):
                """Output projection for chunk j. Yields between tiles.

                Results stage into a per-chunk SBUF tile; one DMA per chunk.
                (The last-processed chunk's projection is merged into the
                finale attention pair instead - see gen_B_pp.)
                """
                ost = outst.tile([128, 4, 2, 512], BF16, tag="ost", name=f"ost{j}")
                for i in range(4):
                    qt_i = j * 4 + i
                    for co in range(2):
                        pj = psA.tile([128, TCH], F32, tag="mm", name=f"pj{j}{i}{co}")
                        for cc in range(2):
                            nc.tensor.matmul(
                                pj[:],
                                ytsb[:, cc, ts(qt_i, 128)],
                                wpj_sb[:, cc, ts(co, TCH)],
                                start=(cc == 0),
                                stop=(cc == 1),
                            )
                        # endgame chunk (exp stream winding down): alternate
                        # DVE/ACT (Copy shares the exp act table - no swaps)
                        # so the copies pipeline two-wide behind the MM chain
                        if j == 3 and co:
                            nc.scalar.copy(ost[:, i, co, :], pj[:])
                        else:
                            nc.vector.tensor_copy(ost[:, i, co, :], pj[:])
                        yield
                nc.sync.dma_start(y[:, j], ost[:])
                yield

            def drain(g):
                for _ in g:
                    pass

            # ---- input DMA schedule ----
            # sync queue: x chunk 0 fine-grained (first matmuls consume
            # per-cc), then chunk 1. Chunks 2/3 go LAST on the scalar queue
            # so their large transfers don't sit ahead of the consts in the
            # DMA-engine queue during the prologue (they aren't consumed
            # until ~25us in).
            # even x0 slices on the sync (HWDGE) queue, odd ones + the
            # later chunks on the pool (SWDGE) queue - SWDGE issues bypass
            # the single shared HWDGE, which otherwise paces the prologue
            # at ~630ns per DMA issue.
            for _cc in range(0, 8, 2):
                nc.sync.dma_start(xt_all[:, 0, _cc, :], xt_d[:, 0, _cc, :])
            nc.gpsimd.dma_start(wv_sb[:], wv[:])
            for _cc in range(1, 8, 2):
                nc.gpsimd.dma_start(xt_all[:, 0, _cc, :], xt_d[:, 0, _cc, :])
            nc.gpsimd.dma_start(xt_all[:, 1], xt_d[:, 1])
            # V ones/zeros columns: needed by the first AV (~25us in), so
            # they sit behind the prologue x DMAs on the pool queue
            nc.gpsimd.memset(vse[:, :, :, 64], 1.0)
            nc.gpsimd.memset(vse[:, :, :, 65:96], 0.0)
            nc.gpsimd.memset(vso[:, :, :, 0], 1.0)
            nc.gpsimd.memset(vso[:, :, :, 1:64], 0.0)
            # scalar (Activation) queue: wqk first (first matmul dep), then
            # chunk-0 rope consts, then the rest. All are prologue-only; the
            # exp stream starts later.
            nc.scalar.dma_start(wqk_sb[:, 0], wqk[:, 0])
            nc.scalar.dma_start(sin_sb[:, ts(0, TCH)], sinp[:, ts(0, TCH)])
            nc.scalar.dma_start(cos_sb[:, ts(0, TCH)], cosp[:, ts(0, TCH)])
            nc.scalar.dma_start(rm_sb[:], rm[:])
            for _pk in range(1, 4):
                nc.scalar.dma_start(wqk_sb[:, _pk], wqk[:, _pk])
            nc.scalar.dma_start(sin_sb[:, TCH:], sinp[:, TCH:])
            nc.scalar.dma_start(cos_sb[:, TCH:], cosp[:, TCH:])
            nc.scalar.dma_start(wpj_sb[:], wpj[:])
            for _c in range(2, NJ):
                nc.gpsimd.dma_start(xt_all[:, _c], xt_d[:, _c])

            # software-pipelined emission at head-pair granularity.
            # filler steps (projections for upcoming chunks + output
            # projections of completed ones) are paced evenly across the
            # attention blocks so the in-order PE always has non-exp-gated
            # work nearby. chunk order 1,2,3,0: the final chunk processed is
            # the one with the SHORTEST attention block, and B(j, pp) only
            # needs pair pp's Q/K packs of chunks 0..j - so pair-1 packs
            # weave into B(j, 0) and the whole prologue exposes just the
            # pair-0 projections of chunks 0 and 1.
            def run_B(b_gen, nb, fills):
                # fills: (iterator, max_steps[, deadline_block]). Items with
                # a deadline are force-drained once that block is reached -
                # a correctness guarantee (e.g. an A-tail's K-pack rotation
                # MUST be emitted before the first score matmul that reads
                # it), not just pacing.
                fills = [f if len(f) == 3 else (f[0], f[1], None) for f in fills]
                total = sum(n for _, n, _ in fills)
                taken = 0
                for k, _ in enumerate(b_gen):
                    for idx, (it, n, dl) in enumerate(fills):
                        if dl is not None and k >= dl:
                            drain(it)
                            taken += n
                            fills[idx] = (it, 0, None)
                    target = min(total, (total * (k + 1) + nb - 1) // nb)
                    while taken < target and fills:
                        it, n, dl = fills[0]
                        if n <= 0:
                            fills.pop(0)
                            continue
                        try:
                            next(it)
                            taken += 1
                            fills[0] = (it, n - 1, dl)
                        except StopIteration:
                            fills.pop(0)
                for it, _, _ in fills:
                    drain(it)

            def nb_of(j, finale=False):
                return 4 * j + 4 + (4 if finale else 1)

            def take(g, n):
                for _ in range(n):
                    try:
                        next(g)
                    except StopIteration:
                        break

            # A_pp(j, pp)'s first two steps (the Q pack) must precede
            # B(j, pp); the rest (K pack + V tiles, first needed at
            # attention block 4j) weaves into B(j, pp)'s own front - run_B
            # consumes fills in list order, so self-tails go first.
            BIG = 10**6
            c0 = None
            for r in range(reps):
                if r == 0:
                    drain(gen_A_pp(0, 0))
                    a10 = gen_A_pp(1, 0)
                    take(a10, 3)
                a01, a11 = gen_A_pp(0, 1), gen_A_pp(1, 1)
                a20, a21 = gen_A_pp(2, 0), gen_A_pp(2, 1)
                a30, a31 = gen_A_pp(3, 0), gen_A_pp(3, 1)
                run_B(gen_B_pp(1, 0), nb_of(1),
                      [(a10, BIG, 4), (a01, BIG), (a11, BIG)])
                take(a20, 3)
                # previous rep's C(0): its normalize input is long ready
                run_B(gen_B_pp(1, 1), nb_of(1),
                      ([(c0, BIG)] if c0 else []) + [(a20, s["a20_b11"])])
                run_B(gen_B_pp(2, 0), nb_of(2),
                      [(a20, BIG, 8), (a21, BIG)])
                c1 = gen_C(1)
                take(a30, 3)
                run_B(gen_B_pp(2, 1), nb_of(2),
                      [(a30, s["a30_b21"]), (c1, s["c1_b21"])])
                run_B(gen_B_pp(3, 0), nb_of(3),
                      [(a30, BIG, 12), (a31, BIG), (c1, BIG)])
                c2 = gen_C(2)
                run_B(gen_B_pp(3, 1), nb_of(3), [(c2, s["c2_b31"])])
                c3 = gen_C(3)
                last_rep = r + 1 >= reps
                run_B(gen_B_pp(0, 0), nb_of(0),
                      [(c2, BIG), (c3, s["c3_b00"])])
                if last_rep:
                    run_B(gen_B_pp(0, 1, finale=True), nb_of(0, True),
                          [(c3, BIG)])
                    c0 = None
                else:
                    a0n, a1n = gen_A_pp(0, 0), gen_A_pp(1, 0)
                    run_B(gen_B_pp(0, 1), nb_of(0), [(c3, BIG), (a0n, BIG)])
                    take(a1n, 3)
                    c0 = gen_C(0)
                    a10 = a1n
            if c0:
                drain(c0)

    nc.compile()
    return nc


@lru_cache(maxsize=8)
def _get_nc(reps=1, variant=""):
    return _build_nc(reps, variant)


def _host_tables():
    dh = DH
    invf = 1.0 / (ROPE_BASE ** (np.arange(0, dh, 2, dtype=np.float64) / dh))
    t = np.arange(T, dtype=np.float64)
    freqs = np.outer(t, invf)  # [T, 32]
    cos_td = np.repeat(np.cos(freqs), 2, axis=1)  # [T, 64]
    sin_td = np.repeat(np.sin(freqs), 2, axis=1)
    import ml_dtypes

    cosp = np.tile(cos_td.T, (2, 1)).astype(ml_dtypes.bfloat16)  # [128, T]
    sinp = np.tile(sin_td.T, (2, 1)).astype(ml_dtypes.bfloat16)

    r = np.zeros((64, 64), dtype=np.float32)
    for i in range(32):
        r[2 * i, 2 * i + 1] = -1.0
        r[2 * i + 1, 2 * i] = 1.0
    r2 = np.zeros((128, 128), dtype=np.float32)
    r2[0:64, 0:64] = r
    r2[64:128, 64:128] = r
    r2t = np.ascontiguousarray(r2.T)

    rr = np.arange(128)[:, None]
    cc = np.arange(128)[None, :]
    maskt = (rr <= cc).astype(np.float32)
    return cosp, sinp, r2t, maskt


def _in_maps(x, Wqkv, Wproj):
    import ml_dtypes

    cosp, sinp, r2t, maskt = _host_tables()
    rm = np.stack([r2t, maskt], axis=1).astype(ml_dtypes.bfloat16)  # [128, 2, 128]
    maps = []
    for c in range(NCORES):
        b, g = divmod(c, 4)
        # xt [128, NJ, 8, TCH]: xt[p, j, cc, t] = x[b, j*TCH + t, cc*128 + p]
        xT = np.ascontiguousarray(x[b].T)  # [C, T]
        xt = (
            xT.reshape(8, 128, NJ, TCH)
            .transpose(1, 2, 0, 3)
            .astype(ml_dtypes.bfloat16)
        )
        wq = Wqkv[:, g * 256:(g + 1) * 256]
        wk = Wqkv[:, C + g * 256: C + (g + 1) * 256]
        wvv = Wqkv[:, 2 * C + g * 256: 2 * C + (g + 1) * 256]
        # wqk [128, 4, 8, 128]: wqk[p, pk, cc, m] = cat(wq, wk)[cc*128+p,
        #                                                       pk*128+m]
        wqkc = np.concatenate([wq, wk], axis=1).reshape(8, 128, 4, 128)
        wqkc = np.ascontiguousarray(
            wqkc.transpose(1, 2, 0, 3), dtype=ml_dtypes.bfloat16
        )
        wvc = wvv.reshape(8, 128, 256)
        wvc = np.ascontiguousarray(wvc.transpose(1, 0, 2), dtype=ml_dtypes.bfloat16)
        wp = Wproj[g * 256:(g + 1) * 256, :].reshape(2, 128, 1024)
        wpc = np.ascontiguousarray(wp.transpose(1, 0, 2), dtype=ml_dtypes.bfloat16)
        maps.append(
            {
                "xt": xt,
                "wqk": wqkc,
                "wv": wvc,
                "wpj": wpc,
                "rm": rm,
                "cosp": cosp,
                "sinp": sinp,
            }
        )
    return maps


def _assemble(results):
    out = np.zeros((B, T, C), dtype=np.float32)
    for c in range(NCORES):
        b = c // 4
        # y [128, NJ, 4, 2, 512]: y[p, j, i, co, k] = out[b, j*512+i*128+p,
        #                                                  co*512+k]
        yd = np.asarray(results[c]["y"], dtype=np.float32)
        yr = yd.transpose(1, 2, 0, 3, 4).reshape(T, C)
        out[b] += yr
    return out


def kernel(x, Wqkv, Wproj):
    x = np.asarray(x, dtype=np.float32)
    Wqkv = np.asarray(Wqkv, dtype=np.float32)
    Wproj = np.asarray(Wproj, dtype=np.float32)
    nc = _get_nc()
    maps = _in_maps(x, Wqkv, Wproj)
    res = run_bass_kernel_spmd(nc, maps, core_ids=list(range(NCORES)))
    return _assemble(res.results)
